# revision 4
# baseline (speedup 1.0000x reference)
"""Trainium2 Bass kernel for nn_BitwiseTasNet — v2: M-form collapse.

Because the residual blocks have NO nonlinearity, the depthwise dilated conv
commutes through the 1x1 convs:

    r = W2 @ (sum_k Wk * shift_k(W1 @ h))  =  sum_k M_k @ shift_k(h),
    M_k = W2 . diag(a3 wd_k a2) . W1 . diag(a1)   [C x C, host-folded]

so each block is just 3 small C x C GEMMs on shifted views of h (3C^2 < 2CD:
25% fewer MACs than the reference structure) accumulated straight into a
persistent fp32 PSUM residual. No depthwise taps, no t-evictions, no
DVE/Pool work in the block loop at all. All affine/bias/edge effects are
propagated host-side into a constant field f [C, L] added before the sigmoid.

Per block: 36 matmuls (3 taps x 2 c-steps x 2 mc x 3 free-chunks) + 4 ScalarE
PSUM->SBUF evictions (640/163 split so evictions overlap the PE's tail chunks
and the next block's matmuls never wait). hb is ping-pong double-buffered to
avoid WAR stalls. Sharding: data-parallel over batch N=4 on 4 cores.
"""
import sys
import numpy as np
import ml_dtypes

sys.path.insert(0, "/opt/trn_rl_repo")

from concourse import bass, bacc, tile, mybir  # noqa: E402
from concourse.bass_utils import run_bass_kernel_spmd  # noqa: E402

N, CIN, T = 4, 1, 8000
C, D = 256, 512
FK, FS = 20, 10
BLOCKS, NB = 8, 32
EPS = 1e-5
L = 803
PAD = 128
TW = PAD + 804 + PAD          # hb tile width (zero halos)
ECH = [(0, 512), (512, L)]    # encoder / decoder chunking
MCH = [(0, 512), (512, 640), (640, L)]   # block matmul free-dim chunks
EVA, EVB = (0, 640), (640, L)            # eviction split

F32 = mybir.dt.float32
BF16 = mybir.dt.bfloat16
bf16 = ml_dtypes.bfloat16
AF = mybir.ActivationFunctionType


# ----------------------------------------------------------------- host math
def shiftZ(x, s):
    out = np.zeros_like(x)
    if s >= 0:
        out[:, :L - s] = x[:, s:]
    else:
        out[:, -s:] = x[:, :L + s]
    return out


def fold_params(inp):
    p = {k: np.asarray(v, dtype=np.float64) for k, v in inp.items()}
    a = {}
    for nm in ('bn1', 'bn2', 'bn3'):
        sc = p[nm + '_g'] / np.sqrt(p[nm + '_v'] + EPS)
        sh = p[nm + '_b'] - p[nm + '_m'] * sc
        a[nm] = (sc, sh)
    a1, c1 = a['bn1']; a2, c2 = a['bn2']; a3, c3 = a['bn3']
    w1 = p['w1'][:, :, :, 0]
    W2 = p['w2'][:, :, :, 0]
    Wk = a3[:, None, :] * np.transpose(p['wd'][:, :, 0, :], (0, 2, 1))
    W1a = a2[:, :, None] * w1 * a1[:, None, :]
    b = a2 * np.einsum('idc,ic->id', w1, c1) + c2
    M = np.einsum('icd,ikd,ide->ikce', W2, Wk, W1a, optimize=True)
    beta2 = np.einsum('icd,id->ic', W2, c3)
    f = np.zeros((C, L))
    for i in range(NB):
        d = 2 ** (i % BLOCKS)
        lin = sum(M[i, k] @ shiftZ(f, (k - 1) * d) for k in range(3))
        tb = np.broadcast_to(b[i][:, None], (D, L))
        cimg = sum(W2[i] @ shiftZ(Wk[i, k][:, None] * tb, (k - 1) * d)
                   for k in range(3))
        f = f + lin + cimg + beta2[i][:, None]
    return dict(M=M, f=f, Wenc=p['w_enc'][:, 0, :], Wdec=p['w_dec'][:, 0, :])


def im2col(x):
    xp = np.zeros((N, T + 2 * FK), dtype=np.float32)
    xp[:, FK:FK + T] = np.asarray(x, np.float32)[:, 0, :]
    idx = FS * np.arange(L)[None, :] + np.arange(FK)[:, None]
    return xp[:, idx]


def pack_host(f):
    # stationaries: mw[i][:, ((k*2+cs)*2+mc)*128 + j] = M[i,k][mc*128+j, cs*128+pc]
    mw = np.zeros((NB, 128, 12 * 128), np.float32)
    for k in range(3):
        for cs in range(2):
            for mc in range(2):
                blk = f['M'][:, k, mc * 128:(mc + 1) * 128,
                             cs * 128:(cs + 1) * 128]          # [NB,128o,128c]
                mw[:, :, ((k * 2 + cs) * 2 + mc) * 128:
                   ((k * 2 + cs) * 2 + mc + 1) * 128] = \
                    np.transpose(blk, (0, 2, 1))
    wenct = f['Wenc'].T.astype(np.float32)                     # [20, 256]
    wdect = np.zeros((128, 40), np.float32)
    for k in range(2):
        wdect[:, k * 20:(k + 1) * 20] = f['Wdec'][k * 128:(k + 1) * 128, :]
    sft = np.zeros((128, 2 * L), np.float32)
    for mc in range(2):
        sft[:, mc * L:(mc + 1) * L] = f['f'][mc * 128:(mc + 1) * 128]
    return dict(mw=mw.astype(bf16), wenct=wenct.astype(bf16),
                wdect=wdect.astype(bf16), sft=sft.astype(bf16))


# -------------------------------------------------------------- device build
def build_nc(n_cores=4, n_blocks=NB):
    nc = bacc.Bacc("TRN2", target_bir_lowering=False, debug=False,
                   num_devices=n_cores)
    xcol_d = nc.dram_tensor("xcol", [FK, L], BF16, kind="ExternalInput")
    mw_d = nc.dram_tensor("mw", [NB, 128, 12 * 128], BF16,
                          kind="ExternalInput")
    wenc_d = nc.dram_tensor("wenct", [FK, C], BF16, kind="ExternalInput")
    wdec_d = nc.dram_tensor("wdect", [128, 40], BF16, kind="ExternalInput")
    sf_d = nc.dram_tensor("sft", [128, 2 * L], BF16, kind="ExternalInput")
    out_d = nc.dram_tensor("out", [10, 800], F32, kind="ExternalOutput")

    with tile.TileContext(nc) as tc:
        with (
            tc.tile_pool(name="fix", bufs=1) as fix,
            tc.tile_pool(name="mwp", bufs=8) as mwpool,
            tc.tile_pool(name="hpp", bufs=1, space="PSUM") as hpp,
            tc.tile_pool(name="pop", bufs=2, space="PSUM") as pop,
        ):
            xcol = fix.tile([FK, L], BF16, tag="xcol")
            wenc = fix.tile([FK, C], BF16, tag="wenc")
            wdec = fix.tile([128, 40], BF16, tag="wdec")
            sf = fix.tile([128, 2 * L], BF16, tag="sf")
            # ping-pong residual stream with zero halos: hb[par][cs]
            hb = [[fix.tile([128, TW], BF16, tag=f"hb{p}{cs}",
                            name=f"hb{p}{cs}") for cs in range(2)]
                  for p in range(2)]
            xe = [fix.tile([128, L], BF16, tag=f"xe{m}", name=f"xe{m}")
                  for m in range(2)]
            hs = [fix.tile([128, L], BF16, tag=f"hs{m}", name=f"hs{m}")
                  for m in range(2)]
            yy = [fix.tile([128, L], BF16, tag=f"y{m}", name=f"y{m}")
                  for m in range(2)]
            outsb = fix.tile([10, 800], F32, tag="outsb")
            hp = [hpp.tile([128, L], F32, tag=f"hp{mc}", name=f"hp{mc}")
                  for mc in range(2)]

            # encoder inputs first; tail-only tensors go on the DVE queue so
            # the sync queue starts streaming block weights immediately.
            nc.sync.dma_start(out=xcol[:], in_=xcol_d.ap())
            nc.sync.dma_start(out=wenc[:], in_=wenc_d.ap())
            nc.scalar.dma_start(out=wdec[:], in_=wdec_d.ap())
            nc.scalar.dma_start(out=sf[:], in_=sf_d.ap())

            warm = fix.tile([128, 512], BF16, tag="warm")
            nc.gpsimd.memset(warm[:], 0.0)
            for p in range(2):
                for cs in range(2):
                    nc.gpsimd.memset(hb[p][cs][:, 0:PAD], 0.0)
                    nc.gpsimd.memset(hb[p][cs][:, PAD + L:TW], 0.0)

            # PE warmup: ramp the p-state while the first weight DMAs land
            for w in range(9):
                pw = pop.tile([10, 512], F32, tag="po", name="pw")
                nc.tensor.matmul(pw[:], warm[:, 0:10], warm[:],
                                 start=True, stop=True)

            # ---- encoder: seed hp and hb[0] ----
            for mc in range(2):
                for (c0, c1) in ECH:
                    nc.tensor.matmul(
                        hp[mc][:, c0:c1], wenc[:, mc * 128:(mc + 1) * 128],
                        xcol[:, c0:c1], start=True, stop=True)
            for mc in range(2):
                nc.scalar.copy(hb[0][mc][:, PAD:PAD + L], hp[mc][:])
                nc.vector.tensor_copy(xe[mc][:], hb[0][mc][:, PAD:PAD + L])

            # ---- 32 blocks ----
            for i in range(n_blocks):
                d = 2 ** (i % BLOCKS)
                par, nxt = i % 2, (i + 1) % 2
                mw = mwpool.tile([128, 12 * 128], BF16, tag="mw", name="mw")
                nc.sync.dma_start(out=mw[:], in_=mw_d.ap()[i])

                def wsl(k, cs, mc):
                    o = ((k * 2 + cs) * 2 + mc) * 128
                    return mw[:, o:o + 128]

                def mms(mc, chunks):
                    # chunk-inner so each stationary streams all its chunks
                    # back-to-back (next LDWEIGHTS hides behind them);
                    # cs-outer so cs1 reads start late enough for the
                    # previous block's split mc1 evictions to land.
                    for cs in range(2):
                        for k in range(3):
                            s = (k - 1) * d
                            for (c0, c1) in chunks:
                                nc.tensor.matmul(
                                    hp[mc][:, c0:c1], wsl(k, cs, mc),
                                    hb[par][cs][:, PAD + c0 + s:PAD + c1 + s],
                                    start=False,
                                    stop=(k == 2 and cs == 1))

                # PE: cA chunks for mc0, then mc1; evictions split at bank
                # boundaries so next block's early matmuls only wait on the
                # 512-part of each stream half.
                def ev(mc, c0, c1):
                    nc.scalar.copy(hb[nxt][mc][:, PAD + c0:PAD + c1],
                                   hp[mc][:, c0:c1])

                mms(0, MCH[:2])
                mms(1, MCH[:2])
                ev(0, EVA[0], EVA[1])
                mms(0, MCH[2:])
                mms(1, MCH[2:])
                ev(1, EVA[0], EVA[1])
                ev(0, EVB[0], EVB[1])
                # last tail eviction on the otherwise-idle DVE: shortens the
                # Act queue at the block boundary
                nc.vector.tensor_copy(
                    hb[nxt][1][:, PAD + EVB[0]:PAD + EVB[1]],
                    hp[1][:, EVB[0]:EVB[1]])

            # ---- mask + decoder, chunk-aligned so each decoder GEMM fires
            # as soon as its half of the mask is ready (3-col overlap) ----
            fin = n_blocks % 2
            for ci, (c0, c1) in enumerate([(0, 258), (258, 516), (513, L)]):
                for mc in range(2):
                    nc.vector.tensor_add(
                        hs[mc][:, c0:c1], hb[fin][mc][:, PAD + c0:PAD + c1],
                        sf[:, mc * L + c0:mc * L + c1])
                for mc in range(2):
                    nc.scalar.activation(hs[mc][:, c0:c1], hs[mc][:, c0:c1],
                                         AF.Sigmoid)
                for mc in range(2):
                    nc.vector.tensor_mul(yy[mc][:, c0:c1], xe[mc][:, c0:c1],
                                         hs[mc][:, c0:c1])
                if ci == 0:
                    continue
                d0, d1 = (0, 512) if ci == 1 else (512, 800)
                po = pop.tile([10, 512], F32, tag="po", name="po")
                for k in range(2):
                    nc.tensor.matmul(
                        po[:, 0:d1 - d0], wdec[:, k * 20:k * 20 + 10],
                        yy[k][:, d0 + 2:d1 + 2], start=(k == 0), stop=False)
                    nc.tensor.matmul(
                        po[:, 0:d1 - d0], wdec[:, k * 20 + 10:k * 20 + 20],
                        yy[k][:, d0 + 1:d1 + 1], start=False, stop=(k == 1))
                nc.scalar.copy(outsb[:, d0:d1], po[:, 0:d1 - d0])
                nc.sync.dma_start(out=out_d.ap()[:, d0:d1],
                                  in_=outsb[:, d0:d1])

    nc.compile()
    return nc


# ------------------------------------------------------------------- driver
_CACHE = {}


def _get_nc(n_cores, n_blocks):
    key = (n_cores, n_blocks)
    if key not in _CACHE:
        _CACHE[key] = build_nc(n_cores, n_blocks)
    return _CACHE[key]


def run(inputs, n_blocks=NB, trace=False):
    f = fold_params(inputs)
    pk = pack_host(f)
    xc = im2col(inputs['x']).astype(bf16)
    n_cores = 4
    nc = _get_nc(n_cores, n_blocks)
    in_maps = []
    for n in range(n_cores):
        in_maps.append(dict(
            xcol=xc[n], mw=pk['mw'], wenct=pk['wenct'], wdect=pk['wdect'],
            sft=pk['sft']))
    res = run_bass_kernel_spmd(nc, in_maps, list(range(n_cores)), trace=trace)
    out = np.zeros((N, CIN, T), np.float32)
    for n in range(n_cores):
        out[n, 0, :] = res.results[n]['out'].T.reshape(T)
    return out, res


def kernel(**inputs):
    out, _ = run(inputs)
    return out
